# revision 16
# baseline (speedup 1.0000x reference)
"""Multi-head self-attention (B=4, S=4096, D=128, H=4, no scaling, no mask)
on 8 Trainium2 NeuronCores.

Sharding: 16 (batch, head) pairs over 8 cores -> core c handles batch c//2,
heads 2*(c%2) and 2*(c%2)+1. No cross-core communication.

Per-core algorithm (flash-style, scores never touch DRAM), v4:
  - query blocks of 1024; scores psum tiles hold ONE 128-key chunk x 1024
    queries ([128, 1024], 2 banks, bufs=3). One matmul per tile (f32r
    moving at 1 cyc/row, row-tiled via tile_position (32*(j%2), 0), with
    2-replicated q and pair-packed kT). Shorter per-tile emission keeps
    the psum WAR recycle (scores j+3 waits exp j) near the PE roofline.
  - PV SWAPPED: the exp'd scores pt (bf16) are the STATIONARY operand
    ([128 keys x 128 queries] chunks); vhat [128 keys, 33] is the moving
    one. av[128 queries, 8*33] accumulates over all 32 key chunks in one
    psum bank -> only 33 moving rows per (key-chunk, query-chunk) instead
    of 512 (stationary loads are free): ~4x less PE time on PV. Output
    lands in [query, dim] layout, so softmax normalization is per-
    partition scalar ops and the OUT dma is contiguous.
  - av bank opened by a dummy zero matmul (start=True over all 264 cols);
    all real PV matmuls accumulate with start=False (correct under both
    whole-granule and per-byte PSUM zeroing semantics).
  - exp split across ACT (real Exp -> bf16, 18/32) and DVE (Schraudolph
    fast-exp int16(A*s+B) bitcast to bf16, 14/32; ~3% sawtooth error,
    within the 2e-2 tolerance; denominators stay consistent because the
    ones-column sums the same approximated values). Pool/GPSIMD cannot
    access PSUM so it cannot help with the exp.
  - bk is dropped entirely (softmax invariant); bq rides the q evac
    activation; bv rides the DVE vhat bias-add.
  - normalization: DVE reciprocal of the 8 ones-columns, then 4 ACT
    (Identity, scale=rcp) + 4 DVE (tensor_scalar mult) 32-col multiplies.
  - software pipeline: scores(j) emitted; exp(j-1) issued; PV(j-3)
    issued. Projections for xt tiles 1..3 interleave at slots 5/13/21 of
    block 0 with exp pre-issue (avoids psum WAR emission deadlock).
Host gathers OUT [2, S, 32] per core into the full (B, S, D) output.
"""

import sys

for _p in ("/opt/trn_rl_repo", "/root/.axon_site/_ro/trn_rl_repo"):
    if _p not in sys.path:
        sys.path.append(_p)

import numpy as np
from collections import deque
from contextlib import ExitStack

import concourse.bass as bass
import concourse.bacc as bacc
import concourse.mybir as mybir
import concourse.tile as tile
from concourse import bass_utils

F32 = mybir.dt.float32
F32R = mybir.dt.float32r
I32 = mybir.dt.int32
I16 = mybir.dt.int16
BF16 = mybir.dt.bfloat16
AF = mybir.ActivationFunctionType
ALU = mybir.AluOpType

B, D, H, HD = 4, 128, 4, 32
NCORES = 8

# Schraudolph fast-exp in bf16 bit-space: exp(x) ~= bitcast_bf16(int16(A*x+B))
# (bf16 = top 16 bits of f32, so the fp32 constants scale by 2^-16)
LOG2E = 1.4426950408889634
SCH_A = float(np.float32(2.0**7 * LOG2E))
SCH_C = 486411.0 / 2.0**16
SCH_B = float(np.float32(127.0 * 2.0**7 - SCH_C))


def _mk_pat(n, extra_a):
    pat = ["A" if i % 2 == 0 else "D" for i in range(n)]
    for i in extra_a:
        pat[i] = "A"
    return "".join(pat)


# exp engine per chunk slot (A=ACT real exp, D=DVE Schraudolph fast-exp).
# GPSIMD/Pool cannot access PSUM, so only ACT and DVE can evacuate scores.
EXP_PAT = _mk_pat(32, (7, 23))        # ACT 18/32, DVE 14/32
EXP_PAT_B0 = _mk_pat(32, (7, 23))

_built = {}


def build_nc(S):
    """Build + compile the per-core program (identical across cores)."""
    NJ = S // 128    # 128-key chunks
    NQB = S // 1024  # 1024-query blocks per head
    NT = S // 1024   # xt DMA tiles

    nc = bacc.Bacc("TRN2", target_bir_lowering=False, debug=False)

    XT = nc.dram_tensor("XT", [128, S], F32, kind="ExternalInput").ap()
    WBLOB = nc.dram_tensor("WBLOB", [128, 518], F32, kind="ExternalInput").ap()
    OUT = nc.dram_tensor("OUT", [2, S, 32], F32, kind="ExternalOutput").ap()
    # WBLOB cols: 0:128 wq (2-replicated), 128:384 wk (2x2 strided-padded),
    # 384:386 bq, 386:452 wva, 452:518 bvb(+ones)

    with tile.TileContext(nc) as tc, ExitStack() as ctx:
        const = ctx.enter_context(tc.tile_pool(name="const", bufs=1))
        big = ctx.enter_context(tc.tile_pool(name="big", bufs=1))
        pss = ctx.enter_context(tc.tile_pool(name="pss", bufs=3, space="PSUM"))
        psav = ctx.enter_context(tc.tile_pool(name="psav", bufs=2, space="PSUM"))
        work = ctx.enter_context(tc.tile_pool(name="work", bufs=6))
        outp = ctx.enter_context(tc.tile_pool(name="outp", bufs=8))

        # ---- input DMA: weights blob, then xt in NT tiles of 1024 cols
        # Service order on the shared transfer engine: xt0, blobV, blobW,
        # xt1..3 -- tile 0's v-chunks start as soon as xt0+blobV land.
        blobW = const.tile([128, 386], F32R, tag="blobW")
        blobV = const.tile([128, 132], F32R, tag="blobV")
        xts = []
        t0 = big.tile([128, 1024], F32R, tag="xt0", name="xt0")
        nc.sync.dma_start(t0[:], XT[:, 0:1024].bitcast(F32R))
        xts.append(t0)
        nc.sync.dma_start(blobV[:], WBLOB[:, 386:518].bitcast(F32R))
        nc.sync.dma_start(blobW[:], WBLOB[:, 0:386].bitcast(F32R))
        for c in range(1, NT):
            t = big.tile([128, 1024], F32R, tag=f"xt{c}", name=f"xt{c}")
            nc.sync.dma_start(t[:], XT[:, c * 1024 : (c + 1) * 1024].bitcast(F32R))
            xts.append(t)

        # combined-head weights: output partition p = 64h + 32r + e, so one
        # 128-partition matmul projects q (or packs k) for BOTH heads at once
        wq_comb = blobW[:, 0:128]
        wk_comb = [blobW[:, 128 + 128 * r : 128 + 128 * (r + 1)] for r in range(2)]
        bq_comb = blobW[:, 384:385].bitcast(F32)
        wva = blobV[:, 0:66]
        bvb = blobV[:, 66:132].bitcast(F32)

        # persistent activations (rows 64h+32r+e)
        qt_rep = big.tile([128, S], F32R, tag="qt", name="qt")
        kt_pack = big.tile([128, (NJ // 2) * 128], F32R, tag="kt", name="kt")
        # bf16: PV runs fully in bf16 (stationary pt, moving vhat)
        vhat = big.tile([128, NJ * 66], BF16, tag="vhat")

        # bf16 zeros for the av-bank-opening dummy matmul
        zbf = const.tile([128, 512], BF16, tag="zbf")
        nc.vector.memset(zbf[:], 0.0)

        # force the exp_and_others act table (covers identity+exp) up front
        scratch = const.tile([1, 1], F32, tag="scr")
        nc.scalar.activation(scratch[:], blobV[0:1, 0:1].bitcast(F32), AF.Exp)

        # p-state warm-up: ~4.5us of dummy matmuls on zeroed SBUF while the
        # input DMA is in flight, so the real projections start at full PE
        # clock (the ramp needs 3us of contiguous busy)
        zt = const.tile([128, 512], F32, tag="zt")
        nc.vector.memset(zt[:], 0.0)
        ztr = zt.bitcast(F32R)
        zp = pss.tile([128, 1024], F32, tag="s", name="zp")
        for i in range(7):
            nc.tensor.matmul(
                zp[:, 0:512], ztr[:, 0:128], ztr[:, 0:512], start=(i == 0), stop=(i == 6)
            )

        # ---- projection emitters (psum from the pss pool) ----
        def ps_tile(name):
            return pss.tile([128, 1024], F32, tag="s", name=name)

        def v_chunk(j):
            pv = ps_tile(f"pv{j}")
            nc.tensor.matmul(
                pv[:, 0:66],
                xts[j // 8][:, (j % 8) * 128 : (j % 8) * 128 + 128],
                wva,
                start=True,
                stop=True,
            )
            nc.vector.tensor_tensor(
                out=vhat[:, j * 66 : (j + 1) * 66], in0=pv[:, 0:66], in1=bvb, op=ALU.add
            )

        def k_chunk(c):
            # pack kT for chunks 8c..8c+7, both heads: partition 64h+32(j%2)+e,
            # col 128*(j//2)+p
            pk = ps_tile(f"pk{c}")
            xg = xts[c][:].rearrange("d (j p) -> d j p", p=128)
            for r in range(2):
                nc.tensor.matmul(
                    pk[:, 0:512],
                    wk_comb[r],
                    xg[:, r:8:2, :],
                    start=(r == 0),
                    stop=(r == 1),
                )
            # k-mover on ACT so DVE keeps room for the vhat bias adds
            nc.scalar.activation(
                kt_pack[:, c * 512 : (c + 1) * 512],
                pk[:, 0:512],
                AF.Identity,
            )

        def q_chunk(n):
            pq = ps_tile(f"pq{n}")
            nc.tensor.matmul(
                pq[:, 0:512],
                wq_comb,
                xts[n // 2][:, (n % 2) * 512 : (n % 2) * 512 + 512],
                start=True,
                stop=True,
            )
            nc.scalar.activation(
                qt_rep[:, n * 512 : (n + 1) * 512],
                pq[:, 0:512],
                AF.Identity,
                bias=bq_comb,
            )

        def proj_tile(c):
            # k/q first: their movers gate the next scores chunks, while the
            # v-chunk PE work overlaps those movers
            k_chunk(c)
            q_chunk(2 * c)
            q_chunk(2 * c + 1)
            for j in range(8 * c, 8 * c + 8):
                v_chunk(j)

        # ---- attention ----
        # Decoupled software pipeline over "slots": slot s = 2g+half covers
        # key chunks (2g, 2g+1) x 512 queries (half). After scores(s) are
        # emitted, the exp of s-1 is issued and the PV of s-3.
        last_s = NJ - 1
        pending = deque()  # entries: [ps, s, av, h, q0, exp_pt]

        def issue_exp(ent, in_b0):
            ps, s, av, h, q0, _ = ent
            eng = (EXP_PAT_B0 if in_b0 else EXP_PAT)[s]
            if eng == "A":
                ptf = work.tile([128, 1024], BF16, tag="pt", name=f"pt{h}_{q0}_{s}")
                nc.scalar.activation(ptf[:], ps[:], AF.Exp)
                pt = ptf
            else:
                pti = work.tile([128, 1024], I16, tag="pti", name=f"pt{h}_{q0}_{s}")
                nc.vector.tensor_scalar(
                    out=pti[:],
                    in0=ps[:],
                    scalar1=SCH_A,
                    scalar2=SCH_B,
                    op0=ALU.mult,
                    op1=ALU.add,
                )
                pt = pti.bitcast(BF16)
            ent[5] = pt

        def issue_pv():
            ent = pending.popleft()
            if ent[5] is None:
                issue_exp(ent, False)
            _, s, av, h, q0, pt = ent
            g, half = s // 2, s % 2
            for r in range(2):
                j = 2 * g + r
                vs = vhat[:, j * 66 + h * 33 : j * 66 + h * 33 + 33]
                for q2 in range(4):
                    qc = 4 * half + q2
                    nc.tensor.matmul(
                        av[:, qc * 33 : qc * 33 + 33],
                        pt[:, 512 * r + 128 * q2 : 512 * r + 128 * q2 + 128],
                        vs,
                        start=False,
                        stop=(s == last_s and r == 1 and q2 == 3),
                        skip_group_check=True,
                    )
            if s == last_s:
                # normalize straight out of psum: per-partition reciprocal of
                # the ones-columns, then eight 32-col multiplies split over
                # ACT (Identity, scale) and DVE; single contiguous block DMA.
                rcp = outp.tile([128, 8], F32, tag="rcp", name=f"rc{h}_{q0}")
                nc.vector.reciprocal(rcp[:], av[:, 32:264:33])
                osb = outp.tile([128, 256], F32, tag="osb", name=f"ob{h}_{q0}")
                for qc in range(8):
                    if qc % 2 == 0:
                        nc.scalar.activation(
                            osb[:, qc * 32 : qc * 32 + 32],
                            av[:, qc * 33 : qc * 33 + 32],
                            AF.Identity,
                            scale=rcp[:, qc : qc + 1],
                        )
                    else:
                        nc.vector.tensor_scalar(
                            out=osb[:, qc * 32 : qc * 32 + 32],
                            in0=av[:, qc * 33 : qc * 33 + 32],
                            scalar1=rcp[:, qc : qc + 1],
                            scalar2=None,
                            op0=ALU.mult,
                        )
                nc.sync.dma_start(
                    OUT[h, q0 : q0 + 1024, :].rearrange("(c p) d -> p c d", c=8),
                    osb[:].rearrange("p (c d) -> p c d", c=8),
                )

        def on_chunk(in_b0):
            if len(pending) >= 2 and pending[-2][5] is None:
                issue_exp(pending[-2], in_b0)
            if len(pending) >= 4:
                issue_pv()

        def flush_all():
            for ent in pending:
                if ent[5] is None:
                    issue_exp(ent, False)
            while pending:
                issue_pv()

        proj_tile(0)
        for h in range(2):
            for i0 in range(NQB):
                q0 = i0 * 1024
                in_b0 = h == 0 and i0 == 0
                av = psav.tile([128, 512], F32, tag="av", name=f"av{h}_{q0}")
                # open the accumulation bank: zeros over the FULL bank (512
                # cols) in one matmul, as v2 did -- partial-bank openers
                # misbehaved on hardware
                nc.tensor.matmul(
                    av[:, 0:512],
                    zbf[:, 0:128],
                    zbf[:, 0:512],
                    start=True,
                    stop=False,
                    skip_group_check=True,
                )
                for s in range(NJ):
                    g, half = s // 2, s % 2
                    qh0 = q0 + 512 * half
                    ps = pss.tile([128, 1024], F32, tag="s", name=f"s{h}_{q0}_{s}")
                    # slot tile: key chunks (2g, 2g+1) x 512 queries, two
                    # row-band matmuls as in v2 (one psum bank each)
                    for r in range(2):
                        off = 64 * h + 32 * r
                        nc.tensor.matmul(
                            ps[:, 512 * r : 512 * (r + 1)],
                            kt_pack[off : off + 32, g * 128 : g * 128 + 128],
                            qt_rep[off : off + 32, qh0 : qh0 + 512],
                            start=True,
                            stop=True,
                            tile_position=(off, 0),
                        )
                    pending.append([ps, s, av, h, q0, None])
                    on_chunk(in_b0)
                    # interleave remaining xt-tile projections into block 0;
                    # pre-issue pending exps so the 11 psum allocations never
                    # WAR-wait on a not-yet-emitted exp (emission deadlock) —
                    # the PV backlog itself can stay pending.
                    if in_b0 and s in (5, 13, 21):
                        c = s // 8 + 1
                        if c < NT:
                            for ent in pending:
                                if ent[5] is None:
                                    issue_exp(ent, True)
                            proj_tile(c)
        flush_all()

    nc.compile()
    return nc


def _host_prep(x, Wq, bq, Wk, bk, Wv, bv, S):
    """Per-core input maps."""
    in_maps = []
    for c in range(NCORES):
        b, hp = c // 2, c % 2
        h0, h1 = 2 * hp, 2 * hp + 1
        xt = np.ascontiguousarray(x[b].T).astype(np.float32)  # [128, S]
        blob = np.zeros((128, 518), np.float32)
        for i, hh in enumerate((h0, h1)):
            wq_h = Wq[hh * 32 : (hh + 1) * 32, :]  # [32, 128]
            wk_h = Wk[hh * 32 : (hh + 1) * 32, :]
            # combined-head layout: output partition p = 64i + 32r + e
            blob[:, 64 * i : 64 * (i + 1)] = np.tile(wq_h.T, (1, 2))
            for r in range(2):
                off = 128 + 128 * r + 64 * i + 32 * r
                blob[:, off : off + 32] = wk_h.T
            blob[64 * i : 64 * (i + 1), 384] = np.tile(bq[hh * 32 : (hh + 1) * 32], 2)
            blob[:, 386 + 33 * i : 386 + 33 * i + 32] = Wv[hh * 32 : (hh + 1) * 32, :].T
            blob[:, 452 + 33 * i : 452 + 33 * i + 32] = bv[hh * 32 : (hh + 1) * 32][None, :]
            blob[:, 452 + 33 * i + 32] = 1.0
        in_maps.append({"XT": xt, "WBLOB": blob})
    return in_maps


def _unshard(results, S):
    out = np.empty((B, S, D), np.float32)
    for c in range(NCORES):
        b, hp = c // 2, c % 2
        oc = results[c]["OUT"]  # [2, S, 32]
        for hl in range(2):
            hh = 2 * hp + hl
            out[b, :, hh * 32 : (hh + 1) * 32] = oc[hl]
    return out


def _run_once(args):
    x, Wq, bq, Wk, bk, Wv, bv = args
    S = x.shape[1]
    if S not in _built:
        _built[S] = build_nc(S)
    nc = _built[S]
    in_maps = _host_prep(x, Wq, bq, Wk, bk, Wv, bv, S)
    res = bass_utils.run_bass_kernel_spmd(nc, in_maps, core_ids=list(range(NCORES)))
    return _unshard(res.results, S)


def _subproc_entry(args):
    return _run_once(args)


def kernel(x, Wq, bq, Wk, bk, Wv, bv):
    args = tuple(
        np.asarray(a, dtype=np.float32) for a in (x, Wq, bq, Wk, bk, Wv, bv)
    )
    # The axon/NRT stack occasionally fails a first dispatch with
    # NRT_EXEC_UNIT_UNRECOVERABLE (device auto-recovers). Retry in-process,
    # then in a fresh spawned process (compile caches make that cheap).
    try:
        return _run_once(args)
    except Exception:
        try:
            return _run_once(args)
        except Exception:
            import multiprocessing as mp

            ctx = mp.get_context("spawn")
            with ctx.Pool(1) as pool:
                return pool.apply(_subproc_entry, (args,))


# revision 19
# speedup vs baseline: 1.0352x; 1.0352x over previous
"""Multi-head self-attention (B=4, S=4096, D=128, H=4, no scaling, no mask)
on 8 Trainium2 NeuronCores.

Sharding: 16 (batch, head) pairs over 8 cores -> core c handles batch c//2,
heads 2*(c%2) and 2*(c%2)+1. No cross-core communication.

Per-core algorithm (flash-style, scores never touch DRAM), v4:
  - query blocks of 1024; scores psum tiles hold ONE 128-key chunk x 1024
    queries ([128, 1024], 2 banks, bufs=3). One matmul per tile (f32r
    moving at 1 cyc/row, row-tiled via tile_position (32*(j%2), 0), with
    2-replicated q and pair-packed kT). Shorter per-tile emission keeps
    the psum WAR recycle (scores j+3 waits exp j) near the PE roofline.
  - PV SWAPPED: the exp'd scores pt (bf16) are the STATIONARY operand
    ([128 keys x 128 queries] chunks); vhat [128 keys, 33] is the moving
    one. av[128 queries, 8*33] accumulates over all 32 key chunks in one
    psum bank -> only 33 moving rows per (key-chunk, query-chunk) instead
    of 512 (stationary loads are free): ~4x less PE time on PV. Output
    lands in [query, dim] layout, so softmax normalization is per-
    partition scalar ops and the OUT dma is contiguous.
  - av bank opened by a dummy zero matmul (start=True over all 264 cols);
    all real PV matmuls accumulate with start=False (correct under both
    whole-granule and per-byte PSUM zeroing semantics).
  - exp split across ACT (real Exp -> bf16, 18/32) and DVE (Schraudolph
    fast-exp int16(A*s+B) bitcast to bf16, 14/32; ~3% sawtooth error,
    within the 2e-2 tolerance; denominators stay consistent because the
    ones-column sums the same approximated values). Pool/GPSIMD cannot
    access PSUM so it cannot help with the exp.
  - bk is dropped entirely (softmax invariant); bq rides the q evac
    activation; bv rides the DVE vhat bias-add.
  - normalization: DVE reciprocal of the 8 ones-columns, then 4 ACT
    (Identity, scale=rcp) + 4 DVE (tensor_scalar mult) 32-col multiplies.
  - software pipeline: scores(j) emitted; exp(j-1) issued; PV(j-3)
    issued. Projections for xt tiles 1..3 interleave at slots 5/13/21 of
    block 0 with exp pre-issue (avoids psum WAR emission deadlock).
Host gathers OUT [2, S, 32] per core into the full (B, S, D) output.
"""

import sys

for _p in ("/opt/trn_rl_repo", "/root/.axon_site/_ro/trn_rl_repo"):
    if _p not in sys.path:
        sys.path.append(_p)

import numpy as np
from collections import deque
from contextlib import ExitStack

import concourse.bass as bass
import concourse.bacc as bacc
import concourse.mybir as mybir
import concourse.tile as tile
from concourse import bass_utils

F32 = mybir.dt.float32
F32R = mybir.dt.float32r
I32 = mybir.dt.int32
I16 = mybir.dt.int16
BF16 = mybir.dt.bfloat16
AF = mybir.ActivationFunctionType
ALU = mybir.AluOpType

B, D, H, HD = 4, 128, 4, 32
NCORES = 8

# Schraudolph fast-exp in bf16 bit-space: exp(x) ~= bitcast_bf16(int16(A*x+B))
# (bf16 = top 16 bits of f32, so the fp32 constants scale by 2^-16)
LOG2E = 1.4426950408889634
SCH_A = float(np.float32(2.0**7 * LOG2E))
SCH_C = 486411.0 / 2.0**16
SCH_B = float(np.float32(127.0 * 2.0**7 - SCH_C))


def _mk_pat(n, extra_a):
    pat = ["A" if i % 2 == 0 else "D" for i in range(n)]
    for i in extra_a:
        pat[i] = "A"
    return "".join(pat)


# exp engine per chunk slot (A=ACT real exp, D=DVE Schraudolph fast-exp).
# GPSIMD/Pool cannot access PSUM, so only ACT and DVE can evacuate scores.
EXP_PAT = _mk_pat(32, (7,))        # ACT 17/32, DVE 15/32
EXP_PAT_B0 = _mk_pat(32, (7,))

_built = {}


def build_nc(S):
    """Build + compile the per-core program (identical across cores)."""
    NJ = S // 128    # 128-key chunks
    NQB = S // 1024  # 1024-query blocks per head
    NT = S // 1024   # xt DMA tiles

    nc = bacc.Bacc("TRN2", target_bir_lowering=False, debug=False)

    XT = nc.dram_tensor("XT", [128, S], F32, kind="ExternalInput").ap()
    WBLOB = nc.dram_tensor("WBLOB", [128, 518], F32, kind="ExternalInput").ap()
    OUT = nc.dram_tensor("OUT", [2, S, 32], F32, kind="ExternalOutput").ap()
    # WBLOB cols: 0:128 wq (2-replicated), 128:384 wk (2x2 strided-padded),
    # 384:386 bq, 386:452 wva, 452:518 bvb(+ones)

    with tile.TileContext(nc) as tc, ExitStack() as ctx:
        const = ctx.enter_context(tc.tile_pool(name="const", bufs=1))
        big = ctx.enter_context(tc.tile_pool(name="big", bufs=1))
        pss = ctx.enter_context(tc.tile_pool(name="pss", bufs=3, space="PSUM"))
        psav = ctx.enter_context(tc.tile_pool(name="psav", bufs=2, space="PSUM"))
        work = ctx.enter_context(tc.tile_pool(name="work", bufs=6))
        outp = ctx.enter_context(tc.tile_pool(name="outp", bufs=8))

        # ---- input DMA: weights blob, then xt in NT tiles of 1024 cols
        # Service order on the shared transfer engine: xt0, blobV, blobW,
        # xt1..3 -- tile 0's v-chunks start as soon as xt0+blobV land.
        blobW = const.tile([128, 386], F32R, tag="blobW")
        blobV = const.tile([128, 132], F32R, tag="blobV")
        xts = []
        t0 = big.tile([128, 1024], F32R, tag="xt0", name="xt0")
        nc.sync.dma_start(t0[:], XT[:, 0:1024].bitcast(F32R))
        xts.append(t0)
        nc.sync.dma_start(blobV[:], WBLOB[:, 386:518].bitcast(F32R))
        nc.sync.dma_start(blobW[:], WBLOB[:, 0:386].bitcast(F32R))
        for c in range(1, NT):
            t = big.tile([128, 1024], F32R, tag=f"xt{c}", name=f"xt{c}")
            nc.sync.dma_start(t[:], XT[:, c * 1024 : (c + 1) * 1024].bitcast(F32R))
            xts.append(t)

        # combined-head weights: output partition p = 64h + 32r + e, so one
        # 128-partition matmul projects q (or packs k) for BOTH heads at once
        wq_comb = blobW[:, 0:128]
        wk_comb = [blobW[:, 128 + 128 * r : 128 + 128 * (r + 1)] for r in range(2)]
        bq_comb = blobW[:, 384:385].bitcast(F32)
        wva = blobV[:, 0:66]
        bvb = blobV[:, 66:132].bitcast(F32)

        # persistent activations (rows 64h+32r+e)
        qt_rep = big.tile([128, S], F32R, tag="qt", name="qt")
        kt_pack = big.tile([128, (NJ // 2) * 128], F32R, tag="kt", name="kt")
        # bf16: PV runs fully in bf16 (stationary pt, moving vhat)
        vhat = big.tile([128, NJ * 66], BF16, tag="vhat")

        # bf16 zeros for the av-bank-opening dummy matmul
        zbf = const.tile([128, 512], BF16, tag="zbf")
        nc.vector.memset(zbf[:], 0.0)

        # force the exp_and_others act table (covers identity+exp) up front
        scratch = const.tile([1, 1], F32, tag="scr")
        nc.scalar.activation(scratch[:], blobV[0:1, 0:1].bitcast(F32), AF.Exp)

        # p-state warm-up: ~4.5us of dummy matmuls on zeroed SBUF while the
        # input DMA is in flight, so the real projections start at full PE
        # clock (the ramp needs 3us of contiguous busy)
        zt = const.tile([128, 512], F32, tag="zt")
        nc.vector.memset(zt[:], 0.0)
        ztr = zt.bitcast(F32R)
        zp = pss.tile([128, 1024], F32, tag="s", name="zp")
        for i in range(7):
            nc.tensor.matmul(
                zp[:, 0:512], ztr[:, 0:128], ztr[:, 0:512], start=(i == 0), stop=(i == 6)
            )

        # ---- projection emitters (psum from the pss pool) ----
        def ps_tile(name):
            return pss.tile([128, 1024], F32, tag="s", name=name)

        def v_chunk(j):
            pv = ps_tile(f"pv{j}")
            nc.tensor.matmul(
                pv[:, 0:66],
                xts[j // 8][:, (j % 8) * 128 : (j % 8) * 128 + 128],
                wva,
                start=True,
                stop=True,
            )
            nc.vector.tensor_tensor(
                out=vhat[:, j * 66 : (j + 1) * 66], in0=pv[:, 0:66], in1=bvb, op=ALU.add
            )

        def k_chunk(c):
            # pack kT for chunks 8c..8c+7, both heads: partition 64h+32(j%2)+e,
            # col 128*(j//2)+p
            pk = ps_tile(f"pk{c}")
            xg = xts[c][:].rearrange("d (j p) -> d j p", p=128)
            for r in range(2):
                nc.tensor.matmul(
                    pk[:, 0:512],
                    wk_comb[r],
                    xg[:, r:8:2, :],
                    start=(r == 0),
                    stop=(r == 1),
                )
            # k-mover on ACT so DVE keeps room for the vhat bias adds
            nc.scalar.activation(
                kt_pack[:, c * 512 : (c + 1) * 512],
                pk[:, 0:512],
                AF.Identity,
            )

        def q_chunk(n):
            pq = ps_tile(f"pq{n}")
            nc.tensor.matmul(
                pq[:, 0:512],
                wq_comb,
                xts[n // 2][:, (n % 2) * 512 : (n % 2) * 512 + 512],
                start=True,
                stop=True,
            )
            nc.scalar.activation(
                qt_rep[:, n * 512 : (n + 1) * 512],
                pq[:, 0:512],
                AF.Identity,
                bias=bq_comb,
            )

        def proj_tile(c):
            # k/q first: their movers gate the next scores chunks, while the
            # v-chunk PE work overlaps those movers
            k_chunk(c)
            q_chunk(2 * c)
            q_chunk(2 * c + 1)
            for j in range(8 * c, 8 * c + 8):
                v_chunk(j)

        # ---- attention ----
        # Decoupled software pipeline over "slots": slot s = 2g+half covers
        # key chunks (2g, 2g+1) x 512 queries (half). After scores(s) are
        # emitted, the exp of s-1 is issued and the PV of s-3.
        last_s = NJ - 1
        pending = deque()  # entries: [ps, s, av, h, q0, exp_pt]

        def issue_exp(ent, in_b0):
            ps, s, av, h, q0, _ = ent
            eng = (EXP_PAT_B0 if in_b0 else EXP_PAT)[s]
            if eng == "A":
                ptf = work.tile([128, 1024], BF16, tag="pt", name=f"pt{h}_{q0}_{s}")
                nc.scalar.activation(ptf[:], ps[:], AF.Exp)
                pt = ptf
            else:
                pti = work.tile([128, 1024], I16, tag="pti", name=f"pt{h}_{q0}_{s}")
                nc.vector.tensor_scalar(
                    out=pti[:],
                    in0=ps[:],
                    scalar1=SCH_A,
                    scalar2=SCH_B,
                    op0=ALU.mult,
                    op1=ALU.add,
                )
                pt = pti.bitcast(BF16)
            ent[5] = pt

        def issue_pv():
            ent = pending.popleft()
            if ent[5] is None:
                issue_exp(ent, False)
            _, s, av, h, q0, pt = ent
            g, half = s // 2, s % 2
            for r in range(2):
                j = 2 * g + r
                vs = vhat[:, j * 66 + h * 33 : j * 66 + h * 33 + 33]
                for q2 in range(4):
                    qc = 4 * half + q2
                    nc.tensor.matmul(
                        av[:, qc * 33 : qc * 33 + 33],
                        pt[:, 512 * r + 128 * q2 : 512 * r + 128 * q2 + 128],
                        vs,
                        start=False,
                        stop=(s == last_s and r == 1 and q2 == 3),
                        skip_group_check=True,
                    )
            if s == last_s:
                # normalize: per-partition reciprocal of the ones-columns,
                # then eight 32-col multiplies. Steady state: stage av to
                # SBUF by DMA and multiply on the otherwise-idle POOL engine
                # (it cannot read psum); keeps ACT/DVE free for the exp.
                # Final block: engines are idle at the drain, so do it
                # engine-side and skip the DMA hop latency.
                fast_tail = h == 1 and q0 == S - 1024
                osb = outp.tile([128, 256], F32, tag="osb", name=f"ob{h}_{q0}")
                rcp = outp.tile([128, 8], F32, tag="rcp", name=f"rc{h}_{q0}")
                if fast_tail:
                    nc.vector.reciprocal(rcp[:], av[:, 32:264:33])
                    for qc in range(8):
                        if qc % 2 == 0:
                            nc.scalar.activation(
                                osb[:, qc * 32 : qc * 32 + 32],
                                av[:, qc * 33 : qc * 33 + 32],
                                AF.Identity,
                                scale=rcp[:, qc : qc + 1],
                            )
                        else:
                            nc.vector.tensor_scalar(
                                out=osb[:, qc * 32 : qc * 32 + 32],
                                in0=av[:, qc * 33 : qc * 33 + 32],
                                scalar1=rcp[:, qc : qc + 1],
                                scalar2=None,
                                op0=ALU.mult,
                            )
                else:
                    avs = outp.tile([128, 264], F32, tag="avs", name=f"as{h}_{q0}")
                    nc.scalar.activation(avs[:], av[:, 0:264], AF.Identity)
                    nc.vector.reciprocal(rcp[:], avs[:, 32:264:33])
                    for qc in range(8):
                        nc.gpsimd.tensor_scalar(
                            out=osb[:, qc * 32 : qc * 32 + 32],
                            in0=avs[:, qc * 33 : qc * 33 + 32],
                            scalar1=rcp[:, qc : qc + 1],
                            scalar2=None,
                            op0=ALU.mult,
                        )
                nc.sync.dma_start(
                    OUT[h, q0 : q0 + 1024, :].rearrange("(c p) d -> p c d", c=8),
                    osb[:].rearrange("p (c d) -> p c d", c=8),
                )

        def on_chunk(in_b0):
            if pending and pending[-1][5] is None:
                issue_exp(pending[-1], in_b0)
            if len(pending) >= 4:
                issue_pv()

        def flush_all():
            for ent in pending:
                if ent[5] is None:
                    issue_exp(ent, False)
            while pending:
                issue_pv()

        proj_tile(0)
        for h in range(2):
            for i0 in range(NQB):
                q0 = i0 * 1024
                in_b0 = h == 0 and i0 == 0
                av = psav.tile([128, 512], F32, tag="av", name=f"av{h}_{q0}")
                # open the accumulation bank: zeros over the FULL bank (512
                # cols) in one matmul, as v2 did -- partial-bank openers
                # misbehaved on hardware
                nc.tensor.matmul(
                    av[:, 0:512],
                    zbf[:, 0:128],
                    zbf[:, 0:512],
                    start=True,
                    stop=False,
                    skip_group_check=True,
                )
                for s in range(NJ):
                    g, half = s // 2, s % 2
                    qh0 = q0 + 512 * half
                    ps = pss.tile([128, 1024], F32, tag="s", name=f"s{h}_{q0}_{s}")
                    # slot tile: key chunks (2g, 2g+1) x 512 queries, two
                    # row-band matmuls as in v2 (one psum bank each)
                    for r in range(2):
                        off = 64 * h + 32 * r
                        nc.tensor.matmul(
                            ps[:, 512 * r : 512 * (r + 1)],
                            kt_pack[off : off + 32, g * 128 : g * 128 + 128],
                            qt_rep[off : off + 32, qh0 : qh0 + 512],
                            start=True,
                            stop=True,
                            tile_position=(off, 0),
                        )
                    pending.append([ps, s, av, h, q0, None])
                    on_chunk(in_b0)
                    # interleave remaining xt-tile projections into block 0;
                    # pre-issue pending exps so the 11 psum allocations never
                    # WAR-wait on a not-yet-emitted exp (emission deadlock) —
                    # the PV backlog itself can stay pending.
                    if in_b0 and s in (5, 13, 21):
                        c = s // 8 + 1
                        if c < NT:
                            for ent in pending:
                                if ent[5] is None:
                                    issue_exp(ent, True)
                            proj_tile(c)
        flush_all()

    nc.compile()
    return nc


def _host_prep(x, Wq, bq, Wk, bk, Wv, bv, S):
    """Per-core input maps."""
    in_maps = []
    for c in range(NCORES):
        b, hp = c // 2, c % 2
        h0, h1 = 2 * hp, 2 * hp + 1
        xt = np.ascontiguousarray(x[b].T).astype(np.float32)  # [128, S]
        blob = np.zeros((128, 518), np.float32)
        for i, hh in enumerate((h0, h1)):
            wq_h = Wq[hh * 32 : (hh + 1) * 32, :]  # [32, 128]
            wk_h = Wk[hh * 32 : (hh + 1) * 32, :]
            # combined-head layout: output partition p = 64i + 32r + e
            blob[:, 64 * i : 64 * (i + 1)] = np.tile(wq_h.T, (1, 2))
            for r in range(2):
                off = 128 + 128 * r + 64 * i + 32 * r
                blob[:, off : off + 32] = wk_h.T
            blob[64 * i : 64 * (i + 1), 384] = np.tile(bq[hh * 32 : (hh + 1) * 32], 2)
            blob[:, 386 + 33 * i : 386 + 33 * i + 32] = Wv[hh * 32 : (hh + 1) * 32, :].T
            blob[:, 452 + 33 * i : 452 + 33 * i + 32] = bv[hh * 32 : (hh + 1) * 32][None, :]
            blob[:, 452 + 33 * i + 32] = 1.0
        in_maps.append({"XT": xt, "WBLOB": blob})
    return in_maps


def _unshard(results, S):
    out = np.empty((B, S, D), np.float32)
    for c in range(NCORES):
        b, hp = c // 2, c % 2
        oc = results[c]["OUT"]  # [2, S, 32]
        for hl in range(2):
            hh = 2 * hp + hl
            out[b, :, hh * 32 : (hh + 1) * 32] = oc[hl]
    return out


def _run_once(args):
    x, Wq, bq, Wk, bk, Wv, bv = args
    S = x.shape[1]
    if S not in _built:
        _built[S] = build_nc(S)
    nc = _built[S]
    in_maps = _host_prep(x, Wq, bq, Wk, bk, Wv, bv, S)
    res = bass_utils.run_bass_kernel_spmd(nc, in_maps, core_ids=list(range(NCORES)))
    return _unshard(res.results, S)


def _subproc_entry(args):
    return _run_once(args)


def kernel(x, Wq, bq, Wk, bk, Wv, bv):
    args = tuple(
        np.asarray(a, dtype=np.float32) for a in (x, Wq, bq, Wk, bk, Wv, bv)
    )
    # The axon/NRT stack occasionally fails a first dispatch with
    # NRT_EXEC_UNIT_UNRECOVERABLE (device auto-recovers). Retry in-process,
    # then in a fresh spawned process (compile caches make that cheap).
    try:
        return _run_once(args)
    except Exception:
        try:
            return _run_once(args)
        except Exception:
            import multiprocessing as mp

            ctx = mp.get_context("spawn")
            with ctx.Pool(1) as pool:
                return pool.apply(_subproc_entry, (args,))


# revision 20
# speedup vs baseline: 1.0362x; 1.0009x over previous
"""Multi-head self-attention (B=4, S=4096, D=128, H=4, no scaling, no mask)
on 8 Trainium2 NeuronCores.

Sharding: 16 (batch, head) pairs over 8 cores -> core c handles batch c//2,
heads 2*(c%2) and 2*(c%2)+1. No cross-core communication.

Per-core algorithm (flash-style, scores never touch DRAM), v4:
  - query blocks of 1024; scores psum tiles hold ONE 128-key chunk x 1024
    queries ([128, 1024], 2 banks, bufs=3). One matmul per tile (f32r
    moving at 1 cyc/row, row-tiled via tile_position (32*(j%2), 0), with
    2-replicated q and pair-packed kT). Shorter per-tile emission keeps
    the psum WAR recycle (scores j+3 waits exp j) near the PE roofline.
  - PV SWAPPED: the exp'd scores pt (bf16) are the STATIONARY operand
    ([128 keys x 128 queries] chunks); vhat [128 keys, 33] is the moving
    one. av[128 queries, 8*33] accumulates over all 32 key chunks in one
    psum bank -> only 33 moving rows per (key-chunk, query-chunk) instead
    of 512 (stationary loads are free): ~4x less PE time on PV. Output
    lands in [query, dim] layout, so softmax normalization is per-
    partition scalar ops and the OUT dma is contiguous.
  - av bank opened by a dummy zero matmul (start=True over all 264 cols);
    all real PV matmuls accumulate with start=False (correct under both
    whole-granule and per-byte PSUM zeroing semantics).
  - exp split across ACT (real Exp -> bf16, 18/32) and DVE (Schraudolph
    fast-exp int16(A*s+B) bitcast to bf16, 14/32; ~3% sawtooth error,
    within the 2e-2 tolerance; denominators stay consistent because the
    ones-column sums the same approximated values). Pool/GPSIMD cannot
    access PSUM so it cannot help with the exp.
  - bk is dropped entirely (softmax invariant); bq rides the q evac
    activation; bv rides the DVE vhat bias-add.
  - normalization: DVE reciprocal of the 8 ones-columns, then 4 ACT
    (Identity, scale=rcp) + 4 DVE (tensor_scalar mult) 32-col multiplies.
  - software pipeline: scores(j) emitted; exp(j-1) issued; PV(j-3)
    issued. Projections for xt tiles 1..3 interleave at slots 5/13/21 of
    block 0 with exp pre-issue (avoids psum WAR emission deadlock).
Host gathers OUT [2, S, 32] per core into the full (B, S, D) output.
"""

import sys

for _p in ("/opt/trn_rl_repo", "/root/.axon_site/_ro/trn_rl_repo"):
    if _p not in sys.path:
        sys.path.append(_p)

import numpy as np
from collections import deque
from contextlib import ExitStack

import concourse.bass as bass
import concourse.bacc as bacc
import concourse.mybir as mybir
import concourse.tile as tile
from concourse import bass_utils

F32 = mybir.dt.float32
F32R = mybir.dt.float32r
I32 = mybir.dt.int32
I16 = mybir.dt.int16
BF16 = mybir.dt.bfloat16
AF = mybir.ActivationFunctionType
ALU = mybir.AluOpType

B, D, H, HD = 4, 128, 4, 32
NCORES = 8

# Schraudolph fast-exp in bf16 bit-space: exp(x) ~= bitcast_bf16(int16(A*x+B))
# (bf16 = top 16 bits of f32, so the fp32 constants scale by 2^-16)
LOG2E = 1.4426950408889634
SCH_A = float(np.float32(2.0**7 * LOG2E))
SCH_C = 486411.0 / 2.0**16
SCH_B = float(np.float32(127.0 * 2.0**7 - SCH_C))


def _mk_pat(n, extra_a):
    pat = ["A" if i % 2 == 0 else "D" for i in range(n)]
    for i in extra_a:
        pat[i] = "A"
    return "".join(pat)


# exp engine per chunk slot (A=ACT real exp, D=DVE Schraudolph fast-exp).
# GPSIMD/Pool cannot access PSUM, so only ACT and DVE can evacuate scores.
EXP_PAT = _mk_pat(32, (7,))        # ACT 17/32, DVE 15/32
EXP_PAT_B0 = _mk_pat(32, (7,))

_built = {}


def build_nc(S):
    """Build + compile the per-core program (identical across cores)."""
    NJ = S // 128    # 128-key chunks
    NQB = S // 1024  # 1024-query blocks per head
    NT = S // 1024   # xt DMA tiles

    nc = bacc.Bacc("TRN2", target_bir_lowering=False, debug=False)

    XT = nc.dram_tensor("XT", [128, S], F32, kind="ExternalInput").ap()
    WBLOB = nc.dram_tensor("WBLOB", [128, 518], F32, kind="ExternalInput").ap()
    OUT = nc.dram_tensor("OUT", [2, S, 32], F32, kind="ExternalOutput").ap()
    # WBLOB cols: 0:128 wq (2-replicated), 128:384 wk (2x2 strided-padded),
    # 384:386 bq, 386:452 wva, 452:518 bvb(+ones)

    with tile.TileContext(nc) as tc, ExitStack() as ctx:
        const = ctx.enter_context(tc.tile_pool(name="const", bufs=1))
        big = ctx.enter_context(tc.tile_pool(name="big", bufs=1))
        pss = ctx.enter_context(tc.tile_pool(name="pss", bufs=3, space="PSUM"))
        psav = ctx.enter_context(tc.tile_pool(name="psav", bufs=2, space="PSUM"))
        work = ctx.enter_context(tc.tile_pool(name="work", bufs=6))
        outp = ctx.enter_context(tc.tile_pool(name="outp", bufs=8))

        # ---- input DMA: weights blob, then xt in NT tiles of 1024 cols
        # Service order on the shared transfer engine: xt0, blobV, blobW,
        # xt1..3 -- tile 0's v-chunks start as soon as xt0+blobV land.
        blobW = const.tile([128, 386], F32R, tag="blobW")
        blobV = const.tile([128, 132], F32R, tag="blobV")
        xts = []
        t0 = big.tile([128, 1024], F32R, tag="xt0", name="xt0")
        nc.sync.dma_start(t0[:], XT[:, 0:1024].bitcast(F32R))
        xts.append(t0)
        nc.sync.dma_start(blobV[:], WBLOB[:, 386:518].bitcast(F32R))
        nc.sync.dma_start(blobW[:], WBLOB[:, 0:386].bitcast(F32R))
        for c in range(1, NT):
            t = big.tile([128, 1024], F32R, tag=f"xt{c}", name=f"xt{c}")
            nc.sync.dma_start(t[:], XT[:, c * 1024 : (c + 1) * 1024].bitcast(F32R))
            xts.append(t)

        # combined-head weights: output partition p = 64h + 32r + e, so one
        # 128-partition matmul projects q (or packs k) for BOTH heads at once
        wq_comb = blobW[:, 0:128]
        wk_comb = [blobW[:, 128 + 128 * r : 128 + 128 * (r + 1)] for r in range(2)]
        bq_comb = blobW[:, 384:385].bitcast(F32)
        wva = blobV[:, 0:66]
        bvb = blobV[:, 66:132].bitcast(F32)

        # persistent activations (rows 64h+32r+e)
        qt_rep = big.tile([128, S], F32R, tag="qt", name="qt")
        kt_pack = big.tile([128, (NJ // 2) * 128], F32R, tag="kt", name="kt")
        # bf16: PV runs fully in bf16 (stationary pt, moving vhat)
        vhat = big.tile([128, NJ * 66], BF16, tag="vhat")

        # bf16 zeros for the av-bank-opening dummy matmul
        zbf = const.tile([128, 512], BF16, tag="zbf")
        nc.vector.memset(zbf[:], 0.0)

        # force the exp_and_others act table (covers identity+exp) up front
        scratch = const.tile([1, 1], F32, tag="scr")
        nc.scalar.activation(scratch[:], blobV[0:1, 0:1].bitcast(F32), AF.Exp)

        # p-state warm-up: ~4.5us of dummy matmuls on zeroed SBUF while the
        # input DMA is in flight, so the real projections start at full PE
        # clock (the ramp needs 3us of contiguous busy)
        zt = const.tile([128, 512], F32, tag="zt")
        nc.vector.memset(zt[:], 0.0)
        ztr = zt.bitcast(F32R)
        zp = pss.tile([128, 1024], F32, tag="s", name="zp")
        for i in range(7):
            nc.tensor.matmul(
                zp[:, 0:512], ztr[:, 0:128], ztr[:, 0:512], start=(i == 0), stop=(i == 6)
            )

        # ---- projection emitters (psum from the pss pool) ----
        def ps_tile(name):
            return pss.tile([128, 1024], F32, tag="s", name=name)

        def v_chunk(j):
            pv = ps_tile(f"pv{j}")
            nc.tensor.matmul(
                pv[:, 0:66],
                xts[j // 8][:, (j % 8) * 128 : (j % 8) * 128 + 128],
                wva,
                start=True,
                stop=True,
            )
            nc.vector.tensor_tensor(
                out=vhat[:, j * 66 : (j + 1) * 66], in0=pv[:, 0:66], in1=bvb, op=ALU.add
            )

        def k_chunk(c):
            # pack kT for chunks 8c..8c+7, both heads: partition 64h+32(j%2)+e,
            # col 128*(j//2)+p
            pk = ps_tile(f"pk{c}")
            xg = xts[c][:].rearrange("d (j p) -> d j p", p=128)
            for r in range(2):
                nc.tensor.matmul(
                    pk[:, 0:512],
                    wk_comb[r],
                    xg[:, r:8:2, :],
                    start=(r == 0),
                    stop=(r == 1),
                )
            # k-mover on ACT so DVE keeps room for the vhat bias adds
            nc.scalar.activation(
                kt_pack[:, c * 512 : (c + 1) * 512],
                pk[:, 0:512],
                AF.Identity,
            )

        def q_chunk(n):
            pq = ps_tile(f"pq{n}")
            nc.tensor.matmul(
                pq[:, 0:512],
                wq_comb,
                xts[n // 2][:, (n % 2) * 512 : (n % 2) * 512 + 512],
                start=True,
                stop=True,
            )
            nc.scalar.activation(
                qt_rep[:, n * 512 : (n + 1) * 512],
                pq[:, 0:512],
                AF.Identity,
                bias=bq_comb,
            )

        def proj_tile(c):
            # k/q first: their movers gate the next scores chunks, while the
            # v-chunk PE work overlaps those movers
            k_chunk(c)
            q_chunk(2 * c)
            q_chunk(2 * c + 1)
            for j in range(8 * c, 8 * c + 8):
                v_chunk(j)

        # ---- attention ----
        # Decoupled software pipeline over "slots": slot s = 2g+half covers
        # key chunks (2g, 2g+1) x 512 queries (half). After scores(s) are
        # emitted, the exp of s-1 is issued and the PV of s-3.
        last_s = NJ - 1
        pending = deque()  # entries: [ps, s, av, h, q0, exp_pt]

        def issue_exp(ent, in_b0, split=False):
            ps, s, av, h, q0, _ = ent
            eng = (EXP_PAT_B0 if in_b0 else EXP_PAT)[s]
            if split:
                # drain shortcut: halves on both engines in parallel
                pti = work.tile([128, 1024], I16, tag="pti", name=f"pt{h}_{q0}_{s}")
                nc.scalar.activation(
                    pti[:, 0:512].bitcast(BF16), ps[:, 0:512], AF.Exp
                )
                nc.vector.tensor_scalar(
                    out=pti[:, 512:1024],
                    in0=ps[:, 512:1024],
                    scalar1=SCH_A,
                    scalar2=SCH_B,
                    op0=ALU.mult,
                    op1=ALU.add,
                )
                ent[5] = pti.bitcast(BF16)
                return
            if eng == "A":
                ptf = work.tile([128, 1024], BF16, tag="pt", name=f"pt{h}_{q0}_{s}")
                nc.scalar.activation(ptf[:], ps[:], AF.Exp)
                pt = ptf
            else:
                pti = work.tile([128, 1024], I16, tag="pti", name=f"pt{h}_{q0}_{s}")
                nc.vector.tensor_scalar(
                    out=pti[:],
                    in0=ps[:],
                    scalar1=SCH_A,
                    scalar2=SCH_B,
                    op0=ALU.mult,
                    op1=ALU.add,
                )
                pt = pti.bitcast(BF16)
            ent[5] = pt

        def issue_pv():
            ent = pending.popleft()
            if ent[5] is None:
                issue_exp(ent, False)
            _, s, av, h, q0, pt = ent
            g, half = s // 2, s % 2
            for r in range(2):
                j = 2 * g + r
                vs = vhat[:, j * 66 + h * 33 : j * 66 + h * 33 + 33]
                for q2 in range(4):
                    qc = 4 * half + q2
                    nc.tensor.matmul(
                        av[:, qc * 33 : qc * 33 + 33],
                        pt[:, 512 * r + 128 * q2 : 512 * r + 128 * q2 + 128],
                        vs,
                        start=False,
                        stop=(s == last_s and r == 1 and q2 == 3),
                        skip_group_check=True,
                    )
            if s == last_s:
                # normalize: per-partition reciprocal of the ones-columns,
                # then eight 32-col multiplies. Steady state: stage av to
                # SBUF by DMA and multiply on the otherwise-idle POOL engine
                # (it cannot read psum); keeps ACT/DVE free for the exp.
                # Final block: engines are idle at the drain, so do it
                # engine-side and skip the DMA hop latency.
                fast_tail = h == 1 and q0 == S - 1024
                osb = outp.tile([128, 256], F32, tag="osb", name=f"ob{h}_{q0}")
                rcp = outp.tile([128, 8], F32, tag="rcp", name=f"rc{h}_{q0}")
                if fast_tail:
                    nc.vector.reciprocal(rcp[:], av[:, 32:264:33])
                    for ha in range(2):
                        for qc in range(4 * ha, 4 * ha + 4):
                            if qc % 2 == 0:
                                nc.scalar.activation(
                                    osb[:, qc * 32 : qc * 32 + 32],
                                    av[:, qc * 33 : qc * 33 + 32],
                                    AF.Identity,
                                    scale=rcp[:, qc : qc + 1],
                                )
                            else:
                                nc.vector.tensor_scalar(
                                    out=osb[:, qc * 32 : qc * 32 + 32],
                                    in0=av[:, qc * 33 : qc * 33 + 32],
                                    scalar1=rcp[:, qc : qc + 1],
                                    scalar2=None,
                                    op0=ALU.mult,
                                )
                        nc.sync.dma_start(
                            OUT[h, q0 + 512 * ha : q0 + 512 * (ha + 1), :].rearrange(
                                "(c p) d -> p c d", c=4
                            ),
                            osb[:, 128 * ha : 128 * (ha + 1)].rearrange(
                                "p (c d) -> p c d", c=4
                            ),
                        )
                    return
                else:
                    avs = outp.tile([128, 264], F32, tag="avs", name=f"as{h}_{q0}")
                    nc.scalar.activation(avs[:], av[:, 0:264], AF.Identity)
                    nc.vector.reciprocal(rcp[:], avs[:, 32:264:33])
                    for qc in range(8):
                        nc.gpsimd.tensor_scalar(
                            out=osb[:, qc * 32 : qc * 32 + 32],
                            in0=avs[:, qc * 33 : qc * 33 + 32],
                            scalar1=rcp[:, qc : qc + 1],
                            scalar2=None,
                            op0=ALU.mult,
                        )
                nc.sync.dma_start(
                    OUT[h, q0 : q0 + 1024, :].rearrange("(c p) d -> p c d", c=8),
                    osb[:].rearrange("p (c d) -> p c d", c=8),
                )

        def flush_all():
            for ent in pending:
                if ent[5] is None:
                    issue_exp(ent, False)
            while pending:
                issue_pv()

        proj_tile(0)
        for h in range(2):
            for i0 in range(NQB):
                q0 = i0 * 1024
                in_b0 = h == 0 and i0 == 0
                av = psav.tile([128, 512], F32, tag="av", name=f"av{h}_{q0}")
                # open the accumulation bank: zeros over the FULL bank (512
                # cols) in one matmul, as v2 did -- partial-bank openers
                # misbehaved on hardware
                nc.tensor.matmul(
                    av[:, 0:512],
                    zbf[:, 0:128],
                    zbf[:, 0:512],
                    start=True,
                    stop=False,
                    skip_group_check=True,
                )
                for s in range(NJ):
                    g, half = s // 2, s % 2
                    qh0 = q0 + 512 * half
                    ps = pss.tile([128, 1024], F32, tag="s", name=f"s{h}_{q0}_{s}")
                    # slot tile: key chunks (2g, 2g+1) x 512 queries, two
                    # row-band matmuls as in v2 (one psum bank each)
                    for r in range(2):
                        off = 64 * h + 32 * r
                        nc.tensor.matmul(
                            ps[:, 512 * r : 512 * (r + 1)],
                            kt_pack[off : off + 32, g * 128 : g * 128 + 128],
                            qt_rep[off : off + 32, qh0 : qh0 + 512],
                            start=True,
                            stop=True,
                            tile_position=(off, 0),
                        )
                    pending.append([ps, s, av, h, q0, None])
                    tail_split = h == 1 and i0 == NQB - 1 and s >= NJ - 2
                    if pending[-1][5] is None:
                        issue_exp(pending[-1], in_b0, split=tail_split)
                    if len(pending) >= 4:
                        issue_pv()
                    # interleave remaining xt-tile projections into block 0;
                    # pre-issue pending exps so the 11 psum allocations never
                    # WAR-wait on a not-yet-emitted exp (emission deadlock) —
                    # the PV backlog itself can stay pending.
                    if in_b0 and s in (5, 13, 21):
                        c = s // 8 + 1
                        if c < NT:
                            for ent in pending:
                                if ent[5] is None:
                                    issue_exp(ent, True)
                            proj_tile(c)
        flush_all()

    nc.compile()
    return nc


def _host_prep(x, Wq, bq, Wk, bk, Wv, bv, S):
    """Per-core input maps."""
    in_maps = []
    for c in range(NCORES):
        b, hp = c // 2, c % 2
        h0, h1 = 2 * hp, 2 * hp + 1
        xt = np.ascontiguousarray(x[b].T).astype(np.float32)  # [128, S]
        blob = np.zeros((128, 518), np.float32)
        for i, hh in enumerate((h0, h1)):
            wq_h = Wq[hh * 32 : (hh + 1) * 32, :]  # [32, 128]
            wk_h = Wk[hh * 32 : (hh + 1) * 32, :]
            # combined-head layout: output partition p = 64i + 32r + e
            blob[:, 64 * i : 64 * (i + 1)] = np.tile(wq_h.T, (1, 2))
            for r in range(2):
                off = 128 + 128 * r + 64 * i + 32 * r
                blob[:, off : off + 32] = wk_h.T
            blob[64 * i : 64 * (i + 1), 384] = np.tile(bq[hh * 32 : (hh + 1) * 32], 2)
            blob[:, 386 + 33 * i : 386 + 33 * i + 32] = Wv[hh * 32 : (hh + 1) * 32, :].T
            blob[:, 452 + 33 * i : 452 + 33 * i + 32] = bv[hh * 32 : (hh + 1) * 32][None, :]
            blob[:, 452 + 33 * i + 32] = 1.0
        in_maps.append({"XT": xt, "WBLOB": blob})
    return in_maps


def _unshard(results, S):
    out = np.empty((B, S, D), np.float32)
    for c in range(NCORES):
        b, hp = c // 2, c % 2
        oc = results[c]["OUT"]  # [2, S, 32]
        for hl in range(2):
            hh = 2 * hp + hl
            out[b, :, hh * 32 : (hh + 1) * 32] = oc[hl]
    return out


def _run_once(args):
    x, Wq, bq, Wk, bk, Wv, bv = args
    S = x.shape[1]
    if S not in _built:
        _built[S] = build_nc(S)
    nc = _built[S]
    in_maps = _host_prep(x, Wq, bq, Wk, bk, Wv, bv, S)
    res = bass_utils.run_bass_kernel_spmd(nc, in_maps, core_ids=list(range(NCORES)))
    return _unshard(res.results, S)


def _subproc_entry(args):
    return _run_once(args)


def kernel(x, Wq, bq, Wk, bk, Wv, bv):
    args = tuple(
        np.asarray(a, dtype=np.float32) for a in (x, Wq, bq, Wk, bk, Wv, bv)
    )
    # The axon/NRT stack occasionally fails a first dispatch with
    # NRT_EXEC_UNIT_UNRECOVERABLE (device auto-recovers). Retry in-process,
    # then in a fresh spawned process (compile caches make that cheap).
    try:
        return _run_once(args)
    except Exception:
        try:
            return _run_once(args)
        except Exception:
            import multiprocessing as mp

            ctx = mp.get_context("spawn")
            with ctx.Pool(1) as pool:
                return pool.apply(_subproc_entry, (args,))


# revision 32
# speedup vs baseline: 1.0749x; 1.0374x over previous
"""Multi-head self-attention (B=4, S=4096, D=128, H=4, no scaling, no mask)
on 8 Trainium2 NeuronCores.

Sharding: 16 (batch, head) pairs over 8 cores -> core c handles batch c//2,
heads 2*(c%2) and 2*(c%2)+1. No cross-core communication.

Per-core algorithm (flash-style, scores never touch DRAM), v5:
  - query blocks of 1024. Scores psum tiles [128, 1024] (2 banks, bufs=3)
    cover one slot s = 2g+half: key chunks (2g, 2g+1) x 512 queries, via
    two row-tiled matmuls (tile_position (64h+32r, 0), 2-replicated q,
    pair-packed kT, f32r moving at 1 cyc/row). NOTE: consecutive matmuls
    with the SAME stationary into different psum banks silently corrupt
    on hardware -- every tile pairs two DIFFERENT stationaries.
  - PV SWAPPED: the exp'd scores pt (bf16) are the STATIONARY operand
    ([128 keys x 128 queries] chunks); vhat [128 keys, 33] is the moving
    one. av[128 queries, 8*33] accumulates over all 32 key chunks in ONE
    psum bank -> 33 moving rows per (key chunk, query chunk) instead of
    512 (stationary loads are free): ~4x less PE time on PV than v2, and
    the output lands in [query, dim] layout.
  - the av bank is opened by a full-bank (512 col) zero matmul with
    start=True; all real PV matmuls accumulate with start=False (partial
    -bank openers misbehaved on hardware).
  - exp split across ACT (real Exp -> bf16) and DVE (Schraudolph
    fast-exp int16(A*s+B) bitcast to bf16; ~3% sawtooth, inside the 2e-2
    tolerance; denominators stay consistent because the ones-column sums
    the same approximated values). Pool/GPSIMD cannot access PSUM, so
    only ACT/DVE can evacuate scores: ACT 17/32 slots, DVE 15/32
    (16/16 in block 0, which also carries the tile 1..3 projections).
    The last two slots' exps split across both engines (shorter drain).
  - bk dropped (softmax invariant); bq rides the q evac activation; bv
    rides a broadcast (stride-0) tensor_tensor pair bias-add.
  - normalization per block: DVE reciprocal of the 8 ones-columns
    ([128,8] strided), then ONE broadcast tensor_tensor multiply, then
    one contiguous OUT dma ([2, S, 32] layout needs no host transpose).
  - pipeline: exp(s) issued right after scores(s); PV(s-5) per slot;
    projections for xt tiles 1..3 interleave at slots 5/13/21 of block 0
    with exp pre-issue (avoids psum WAR emission deadlock).
Cost-model timeline: 184.0us vs 240.7us for v2 (PE 149.8us busy, ACT
150.0, DVE 152.2 -- the ACT+DVE exp evacuation of 33.5M scores is the
roofline, PE is within 3% of it).
Host gathers OUT [2, S, 32] per core into the full (B, S, D) output.
"""

import sys

for _p in ("/opt/trn_rl_repo", "/root/.axon_site/_ro/trn_rl_repo"):
    if _p not in sys.path:
        sys.path.append(_p)

import numpy as np
from collections import deque
from contextlib import ExitStack

import concourse.bass as bass
import concourse.bacc as bacc
import concourse.mybir as mybir
import concourse.tile as tile
from concourse import bass_utils

F32 = mybir.dt.float32
F32R = mybir.dt.float32r
I32 = mybir.dt.int32
I16 = mybir.dt.int16
BF16 = mybir.dt.bfloat16
AF = mybir.ActivationFunctionType
ALU = mybir.AluOpType

B, D, H, HD = 4, 128, 4, 32
NCORES = 8

# Schraudolph fast-exp in bf16 bit-space: exp(x) ~= bitcast_bf16(int16(A*x+B))
# (bf16 = top 16 bits of f32, so the fp32 constants scale by 2^-16)
LOG2E = 1.4426950408889634
SCH_A = float(np.float32(2.0**7 * LOG2E))
SCH_C = 486411.0 / 2.0**16
SCH_B = float(np.float32(127.0 * 2.0**7 - SCH_C))


def _mk_pat(n, extra_a):
    pat = ["A" if i % 2 == 0 else "D" for i in range(n)]
    for i in extra_a:
        pat[i] = "A"
    return "".join(pat)


# exp engine per chunk slot (A=ACT real exp, D=DVE Schraudolph fast-exp).
# GPSIMD/Pool cannot access PSUM, so only ACT and DVE can evacuate scores.
EXP_PAT = _mk_pat(32, (7,))        # ACT 17/32, DVE 15/32
EXP_PAT_B0 = _mk_pat(32, (7,))

_built = {}


def build_nc(S):
    """Build + compile the per-core program (identical across cores)."""
    NJ = S // 128    # 128-key chunks
    NQB = S // 1024  # 1024-query blocks per head
    NT = S // 1024   # xt DMA tiles

    nc = bacc.Bacc("TRN2", target_bir_lowering=False, debug=False)

    XT = nc.dram_tensor("XT", [128, S], F32, kind="ExternalInput").ap()
    WBLOB = nc.dram_tensor("WBLOB", [128, 518], F32, kind="ExternalInput").ap()
    OUT = nc.dram_tensor("OUT", [2, S, 32], F32, kind="ExternalOutput").ap()
    # WBLOB cols: 0:128 wq (2-replicated), 128:384 wk (2x2 strided-padded),
    # 384:386 bq, 386:452 wva, 452:518 bvb(+ones)

    with tile.TileContext(nc) as tc, ExitStack() as ctx:
        const = ctx.enter_context(tc.tile_pool(name="const", bufs=1))
        big = ctx.enter_context(tc.tile_pool(name="big", bufs=1))
        pss = ctx.enter_context(tc.tile_pool(name="pss", bufs=3, space="PSUM"))
        psav = ctx.enter_context(tc.tile_pool(name="psav", bufs=2, space="PSUM"))
        work = ctx.enter_context(tc.tile_pool(name="work", bufs=6))
        outp = ctx.enter_context(tc.tile_pool(name="outp", bufs=8))

        # ---- input DMA: weights blob, then xt in NT tiles of 1024 cols
        # Service order on the shared transfer engine: xt0, blobV, blobW,
        # xt1..3 -- tile 0's v-chunks start as soon as xt0+blobV land.
        blobW = const.tile([128, 386], F32R, tag="blobW")
        blobV = const.tile([128, 132], F32R, tag="blobV")
        xts = []
        t0 = big.tile([128, 1024], F32R, tag="xt0", name="xt0")
        nc.sync.dma_start(t0[:], XT[:, 0:1024].bitcast(F32R))
        xts.append(t0)
        nc.sync.dma_start(blobV[:], WBLOB[:, 386:518].bitcast(F32R))
        nc.sync.dma_start(blobW[:], WBLOB[:, 0:386].bitcast(F32R))
        for c in range(1, NT):
            t = big.tile([128, 1024], F32R, tag=f"xt{c}", name=f"xt{c}")
            nc.sync.dma_start(t[:], XT[:, c * 1024 : (c + 1) * 1024].bitcast(F32R))
            xts.append(t)

        # combined-head weights: output partition p = 64h + 32r + e, so one
        # 128-partition matmul projects q (or packs k) for BOTH heads at once
        wq_comb = blobW[:, 0:128]
        wk_comb = [blobW[:, 128 + 128 * r : 128 + 128 * (r + 1)] for r in range(2)]
        bq_comb = blobW[:, 384:385].bitcast(F32)
        wva = blobV[:, 0:66]
        bvb = blobV[:, 66:132].bitcast(F32)

        # persistent activations (rows 64h+32r+e)
        qt_rep = big.tile([128, S], F32R, tag="qt", name="qt")
        kt_pack = big.tile([128, (NJ // 2) * 128], F32R, tag="kt", name="kt")
        # bf16: PV runs fully in bf16 (stationary pt, moving vhat)
        vhat = big.tile([128, NJ * 66], BF16, tag="vhat")

        # p-state warm-up source first so PE starts as early as possible
        zt = const.tile([128, 512], F32, tag="zt")
        nc.vector.memset(zt[:], 0.0)
        ztr = zt.bitcast(F32R)

        # bf16 zeros for the av-bank-opening dummy matmul (on Pool, in
        # parallel with the DVE memset above)
        zbf = const.tile([128, 512], BF16, tag="zbf")
        nc.gpsimd.memset(zbf[:], 0.0)

        # force the exp_and_others act table (covers identity+exp) up front
        scratch = const.tile([1, 1], F32, tag="scr")
        nc.scalar.activation(scratch[:], blobV[0:1, 0:1].bitcast(F32), AF.Exp)
        zp = pss.tile([128, 1024], F32, tag="s", name="zp")
        for i in range(5):
            nc.tensor.matmul(
                zp[:, 0:512], ztr[:, 0:128], ztr[:, 0:512], start=(i == 0), stop=(i == 4)
            )

        # ---- projection emitters (psum from the pss pool) ----
        def ps_tile(name):
            return pss.tile([128, 1024], F32, tag="s", name=name)

        def v_pair(j):
            # chunks j, j+1 into one tile (separate banks, DIFFERENT
            # stationaries - same-stationary bank pairs corrupt on HW),
            # one broadcast bias-add for both
            pv = ps_tile(f"pv{j}")
            for u in range(2):
                nc.tensor.matmul(
                    pv[:, 512 * u : 512 * u + 66],
                    xts[j // 8][:, ((j + u) % 8) * 128 : ((j + u) % 8) * 128 + 128],
                    wva,
                    start=True,
                    stop=True,
                )
            pv3 = pv[:].rearrange("p (u x) -> p u x", u=2)[:, :, 0:66]
            bv3 = bvb.rearrange("p (u x) -> p u x", u=1)
            pv3b, bv3b = bass.broadcast_tensor_aps(pv3, bv3)
            nc.vector.tensor_tensor(
                out=vhat[:, j * 66 : (j + 2) * 66].rearrange("p (u x) -> p u x", u=2),
                in0=pv3b,
                in1=bv3b,
                op=ALU.add,
            )

        def k_chunk(c):
            # pack kT for chunks 8c..8c+7, both heads: partition 64h+32(j%2)+e,
            # col 128*(j//2)+p
            pk = ps_tile(f"pk{c}")
            xg = xts[c][:].rearrange("d (j p) -> d j p", p=128)
            for r in range(2):
                nc.tensor.matmul(
                    pk[:, 0:512],
                    wk_comb[r],
                    xg[:, r:8:2, :],
                    start=(r == 0),
                    stop=(r == 1),
                )
            # k-mover on ACT so DVE keeps room for the vhat bias adds
            nc.scalar.activation(
                kt_pack[:, c * 512 : (c + 1) * 512],
                pk[:, 0:512],
                AF.Identity,
            )

        def q_chunk(n):
            pq = ps_tile(f"pq{n}")
            nc.tensor.matmul(
                pq[:, 0:512],
                wq_comb,
                xts[n // 2][:, (n % 2) * 512 : (n % 2) * 512 + 512],
                start=True,
                stop=True,
            )
            nc.scalar.activation(
                qt_rep[:, n * 512 : (n + 1) * 512],
                pq[:, 0:512],
                AF.Identity,
                bias=bq_comb,
            )

        def proj_tile(c):
            # k/q first: their movers gate the next scores chunks, while the
            # v-chunk PE work overlaps those movers
            k_chunk(c)
            q_chunk(2 * c)
            q_chunk(2 * c + 1)
            for j in range(8 * c, 8 * c + 8, 2):
                v_pair(j)

        # ---- attention ----
        # Decoupled software pipeline over "slots": slot s = 2g+half covers
        # key chunks (2g, 2g+1) x 512 queries (half). After scores(s) are
        # emitted, the exp of s-1 is issued and the PV of s-3.
        last_s = NJ - 1
        pending = deque()  # entries: [ps, s, av, h, q0, exp_pt]

        def issue_exp(ent, in_b0, split=False):
            ps, s, av, h, q0, _ = ent
            eng = (EXP_PAT_B0 if in_b0 else EXP_PAT)[s]
            if split:
                # drain shortcut: halves on both engines in parallel
                pti = work.tile([128, 1024], I16, tag="pti", name=f"pt{h}_{q0}_{s}")
                nc.scalar.activation(
                    pti[:, 0:512].bitcast(BF16), ps[:, 0:512], AF.Exp
                )
                nc.vector.tensor_scalar(
                    out=pti[:, 512:1024],
                    in0=ps[:, 512:1024],
                    scalar1=SCH_A,
                    scalar2=SCH_B,
                    op0=ALU.mult,
                    op1=ALU.add,
                )
                ent[5] = pti.bitcast(BF16)
                return
            if eng == "A":
                ptf = work.tile([128, 1024], BF16, tag="pt", name=f"pt{h}_{q0}_{s}")
                nc.scalar.activation(ptf[:], ps[:], AF.Exp)
                pt = ptf
            else:
                pti = work.tile([128, 1024], I16, tag="pti", name=f"pt{h}_{q0}_{s}")
                nc.vector.tensor_scalar(
                    out=pti[:],
                    in0=ps[:],
                    scalar1=SCH_A,
                    scalar2=SCH_B,
                    op0=ALU.mult,
                    op1=ALU.add,
                )
                pt = pti.bitcast(BF16)
            ent[5] = pt

        def issue_pv():
            ent = pending.popleft()
            if ent[5] is None:
                issue_exp(ent, False)
            _, s, av, h, q0, pt = ent
            g, half = s // 2, s % 2
            for r in range(2):
                j = 2 * g + r
                vs = vhat[:, j * 66 + h * 33 : j * 66 + h * 33 + 33]
                for q2 in range(4):
                    qc = 4 * half + q2
                    nc.tensor.matmul(
                        av[:, qc * 33 : qc * 33 + 33],
                        pt[:, 512 * r + 128 * q2 : 512 * r + 128 * q2 + 128],
                        vs,
                        start=False,
                        stop=(s == last_s and r == 1 and q2 == 3),
                        skip_group_check=True,
                    )
            if s == last_s:
                # normalize straight out of psum on DVE: reciprocal of the 8
                # ones-columns, then ONE broadcast tensor_tensor multiply
                # (stride-0 free dim on the reciprocal operand).
                rcp = outp.tile([128, 8], F32, tag="rcp", name=f"rc{h}_{q0}")
                nc.vector.reciprocal(rcp[:], av[:, 32:264:33])
                osb = outp.tile([128, 256], F32, tag="osb", name=f"ob{h}_{q0}")
                av3 = av[:, 0:264].rearrange("p (c x) -> p c x", x=33)[:, :, 0:32]
                rcp3 = rcp[:].rearrange("p (c x) -> p c x", x=1)
                av3b, rcp3b = bass.broadcast_tensor_aps(av3, rcp3)
                nc.vector.tensor_tensor(
                    out=osb[:].rearrange("p (c d) -> p c d", c=8),
                    in0=av3b,
                    in1=rcp3b,
                    op=ALU.mult,
                )
                nc.sync.dma_start(
                    OUT[h, q0 : q0 + 1024, :].rearrange("(c p) d -> p c d", c=8),
                    osb[:].rearrange("p (c d) -> p c d", c=8),
                )

        def flush_all():
            for ent in pending:
                if ent[5] is None:
                    issue_exp(ent, False)
            while pending:
                issue_pv()

        proj_tile(0)
        for h in range(2):
            for i0 in range(NQB):
                q0 = i0 * 1024
                in_b0 = h == 0 and i0 == 0
                av = psav.tile([128, 512], F32, tag="av", name=f"av{h}_{q0}")
                # open the accumulation bank: zeros over the FULL bank (512
                # cols) in one matmul, as v2 did -- partial-bank openers
                # misbehaved on hardware
                nc.tensor.matmul(
                    av[:, 0:512],
                    zbf[:, 0:128],
                    zbf[:, 0:512],
                    start=True,
                    stop=False,
                    skip_group_check=True,
                )
                for s in range(NJ):
                    g, half = s // 2, s % 2
                    qh0 = q0 + 512 * half
                    ps = pss.tile([128, 1024], F32, tag="s", name=f"s{h}_{q0}_{s}")
                    # slot tile: key chunks (2g, 2g+1) x 512 queries, two
                    # row-band matmuls as in v2 (one psum bank each)
                    for r in range(2):
                        off = 64 * h + 32 * r
                        nc.tensor.matmul(
                            ps[:, 512 * r : 512 * (r + 1)],
                            kt_pack[off : off + 32, g * 128 : g * 128 + 128],
                            qt_rep[off : off + 32, qh0 : qh0 + 512],
                            start=True,
                            stop=True,
                            tile_position=(off, 0),
                        )
                    pending.append([ps, s, av, h, q0, None])
                    tail_split = h == 1 and i0 == NQB - 1 and s >= NJ - 2
                    if pending[-1][5] is None:
                        issue_exp(pending[-1], in_b0, split=tail_split)
                    if len(pending) >= 5:
                        issue_pv()
                    # interleave remaining xt-tile projections into block 0;
                    # pre-issue pending exps so the 11 psum allocations never
                    # WAR-wait on a not-yet-emitted exp (emission deadlock) —
                    # the PV backlog itself can stay pending.
                    if in_b0 and s in (5, 13, 21):
                        c = s // 8 + 1
                        if c < NT:
                            for ent in pending:
                                if ent[5] is None:
                                    issue_exp(ent, True)
                            proj_tile(c)
        flush_all()

    nc.compile()
    return nc


def _host_prep(x, Wq, bq, Wk, bk, Wv, bv, S):
    """Per-core input maps."""
    in_maps = []
    for c in range(NCORES):
        b, hp = c // 2, c % 2
        h0, h1 = 2 * hp, 2 * hp + 1
        xt = np.ascontiguousarray(x[b].T).astype(np.float32)  # [128, S]
        blob = np.zeros((128, 518), np.float32)
        for i, hh in enumerate((h0, h1)):
            wq_h = Wq[hh * 32 : (hh + 1) * 32, :]  # [32, 128]
            wk_h = Wk[hh * 32 : (hh + 1) * 32, :]
            # combined-head layout: output partition p = 64i + 32r + e
            blob[:, 64 * i : 64 * (i + 1)] = np.tile(wq_h.T, (1, 2))
            for r in range(2):
                off = 128 + 128 * r + 64 * i + 32 * r
                blob[:, off : off + 32] = wk_h.T
            blob[64 * i : 64 * (i + 1), 384] = np.tile(bq[hh * 32 : (hh + 1) * 32], 2)
            blob[:, 386 + 33 * i : 386 + 33 * i + 32] = Wv[hh * 32 : (hh + 1) * 32, :].T
            blob[:, 452 + 33 * i : 452 + 33 * i + 32] = bv[hh * 32 : (hh + 1) * 32][None, :]
            blob[:, 452 + 33 * i + 32] = 1.0
        in_maps.append({"XT": xt, "WBLOB": blob})
    return in_maps


def _unshard(results, S):
    out = np.empty((B, S, D), np.float32)
    for c in range(NCORES):
        b, hp = c // 2, c % 2
        oc = results[c]["OUT"]  # [2, S, 32]
        for hl in range(2):
            hh = 2 * hp + hl
            out[b, :, hh * 32 : (hh + 1) * 32] = oc[hl]
    return out


def _run_once(args):
    x, Wq, bq, Wk, bk, Wv, bv = args
    S = x.shape[1]
    if S not in _built:
        _built[S] = build_nc(S)
    nc = _built[S]
    in_maps = _host_prep(x, Wq, bq, Wk, bk, Wv, bv, S)
    res = bass_utils.run_bass_kernel_spmd(nc, in_maps, core_ids=list(range(NCORES)))
    return _unshard(res.results, S)


def _subproc_entry(args):
    return _run_once(args)


def kernel(x, Wq, bq, Wk, bk, Wv, bv):
    args = tuple(
        np.asarray(a, dtype=np.float32) for a in (x, Wq, bq, Wk, bk, Wv, bv)
    )
    # The axon/NRT stack occasionally fails a first dispatch with
    # NRT_EXEC_UNIT_UNRECOVERABLE (device auto-recovers). Retry in-process,
    # then in a fresh spawned process (compile caches make that cheap).
    try:
        return _run_once(args)
    except Exception:
        try:
            return _run_once(args)
        except Exception:
            import multiprocessing as mp

            ctx = mp.get_context("spawn")
            with ctx.Pool(1) as pool:
                return pool.apply(_subproc_entry, (args,))


# revision 35
# speedup vs baseline: 1.0761x; 1.0011x over previous
"""Multi-head self-attention (B=4, S=4096, D=128, H=4, no scaling, no mask)
on 8 Trainium2 NeuronCores.

Sharding: 16 (batch, head) pairs over 8 cores -> core c handles batch c//2,
heads 2*(c%2) and 2*(c%2)+1. No cross-core communication.

Per-core algorithm (flash-style, scores never touch DRAM), v5:
  - query blocks of 1024. Scores psum tiles [128, 1024] (2 banks, bufs=3)
    cover one slot s = 2g+half: key chunks (2g, 2g+1) x 512 queries, via
    two row-tiled matmuls (tile_position (64h+32r, 0), 2-replicated q,
    pair-packed kT, f32r moving at 1 cyc/row). NOTE: consecutive matmuls
    with the SAME stationary into different psum banks silently corrupt
    on hardware -- every tile pairs two DIFFERENT stationaries.
  - PV SWAPPED: the exp'd scores pt (bf16) are the STATIONARY operand
    ([128 keys x 128 queries] chunks); vhat [128 keys, 33] is the moving
    one. av[128 queries, 8*33] accumulates over all 32 key chunks in ONE
    psum bank -> 33 moving rows per (key chunk, query chunk) instead of
    512 (stationary loads are free): ~4x less PE time on PV than v2, and
    the output lands in [query, dim] layout.
  - the av bank is opened by a full-bank (512 col) zero matmul with
    start=True; all real PV matmuls accumulate with start=False (partial
    -bank openers misbehaved on hardware).
  - exp split across ACT (real Exp -> bf16) and DVE (Schraudolph
    fast-exp int16(A*s+B) bitcast to bf16; ~3% sawtooth, inside the 2e-2
    tolerance; denominators stay consistent because the ones-column sums
    the same approximated values). Pool/GPSIMD cannot access PSUM, so
    only ACT/DVE can evacuate scores: ACT 17/32 slots, DVE 15/32
    (16/16 in block 0, which also carries the tile 1..3 projections).
    The last two slots' exps split across both engines (shorter drain).
  - bk dropped (softmax invariant); bq rides the q evac activation; bv
    rides a broadcast (stride-0) tensor_tensor pair bias-add.
  - normalization per block: DVE reciprocal of the 8 ones-columns
    ([128,8] strided), then ONE broadcast tensor_tensor multiply, then
    one contiguous OUT dma ([2, S, 32] layout needs no host transpose).
  - pipeline: exp(s) issued right after scores(s); PV(s-5) per slot;
    projections for xt tiles 1..3 interleave at slots 1/9/17 of block 0
    with exp pre-issue (avoids psum WAR emission deadlock).
Cost-model timeline: 183.8us vs 240.7us for v2 (PE 149.8us busy, ACT
150.0, DVE 152.2 -- the ACT+DVE exp evacuation of 33.5M scores is the
roofline, PE is within 3% of it).
Host gathers OUT [2, S, 32] per core into the full (B, S, D) output.
"""

import sys

for _p in ("/opt/trn_rl_repo", "/root/.axon_site/_ro/trn_rl_repo"):
    if _p not in sys.path:
        sys.path.append(_p)

import numpy as np
from collections import deque
from contextlib import ExitStack

import concourse.bass as bass
import concourse.bacc as bacc
import concourse.mybir as mybir
import concourse.tile as tile
from concourse import bass_utils

F32 = mybir.dt.float32
F32R = mybir.dt.float32r
I32 = mybir.dt.int32
I16 = mybir.dt.int16
BF16 = mybir.dt.bfloat16
AF = mybir.ActivationFunctionType
ALU = mybir.AluOpType

B, D, H, HD = 4, 128, 4, 32
NCORES = 8

# Schraudolph fast-exp in bf16 bit-space: exp(x) ~= bitcast_bf16(int16(A*x+B))
# (bf16 = top 16 bits of f32, so the fp32 constants scale by 2^-16)
LOG2E = 1.4426950408889634
SCH_A = float(np.float32(2.0**7 * LOG2E))
SCH_C = 486411.0 / 2.0**16
SCH_B = float(np.float32(127.0 * 2.0**7 - SCH_C))


def _mk_pat(n, extra_a):
    pat = ["A" if i % 2 == 0 else "D" for i in range(n)]
    for i in extra_a:
        pat[i] = "A"
    return "".join(pat)


# exp engine per chunk slot (A=ACT real exp, D=DVE Schraudolph fast-exp).
# GPSIMD/Pool cannot access PSUM, so only ACT and DVE can evacuate scores.
EXP_PAT = _mk_pat(32, (7,))        # ACT 17/32, DVE 15/32
EXP_PAT_B0 = _mk_pat(32, (7,))

_built = {}


def build_nc(S):
    """Build + compile the per-core program (identical across cores)."""
    NJ = S // 128    # 128-key chunks
    NQB = S // 1024  # 1024-query blocks per head
    NT = S // 1024   # xt DMA tiles

    nc = bacc.Bacc("TRN2", target_bir_lowering=False, debug=False)

    XT = nc.dram_tensor("XT", [128, S], F32, kind="ExternalInput").ap()
    WBLOB = nc.dram_tensor("WBLOB", [128, 518], F32, kind="ExternalInput").ap()
    OUT = nc.dram_tensor("OUT", [2, S, 32], F32, kind="ExternalOutput").ap()
    # WBLOB cols: 0:128 wq (2-replicated), 128:384 wk (2x2 strided-padded),
    # 384:386 bq, 386:452 wva, 452:518 bvb(+ones)

    with tile.TileContext(nc) as tc, ExitStack() as ctx:
        const = ctx.enter_context(tc.tile_pool(name="const", bufs=1))
        big = ctx.enter_context(tc.tile_pool(name="big", bufs=1))
        pss = ctx.enter_context(tc.tile_pool(name="pss", bufs=3, space="PSUM"))
        psav = ctx.enter_context(tc.tile_pool(name="psav", bufs=2, space="PSUM"))
        work = ctx.enter_context(tc.tile_pool(name="work", bufs=6))
        outp = ctx.enter_context(tc.tile_pool(name="outp", bufs=8))

        # ---- input DMA: weights blob, then xt in NT tiles of 1024 cols
        # Service order on the shared transfer engine: xt0, blobV, blobW,
        # xt1..3 -- tile 0's v-chunks start as soon as xt0+blobV land.
        blobW = const.tile([128, 386], F32R, tag="blobW")
        blobV = const.tile([128, 132], F32R, tag="blobV")
        xts = []
        t0 = big.tile([128, 1024], F32R, tag="xt0", name="xt0")
        nc.sync.dma_start(t0[:], XT[:, 0:1024].bitcast(F32R))
        xts.append(t0)
        nc.sync.dma_start(blobV[:], WBLOB[:, 386:518].bitcast(F32R))
        nc.sync.dma_start(blobW[:], WBLOB[:, 0:386].bitcast(F32R))
        for c in range(1, NT):
            t = big.tile([128, 1024], F32R, tag=f"xt{c}", name=f"xt{c}")
            nc.sync.dma_start(t[:], XT[:, c * 1024 : (c + 1) * 1024].bitcast(F32R))
            xts.append(t)

        # combined-head weights: output partition p = 64h + 32r + e, so one
        # 128-partition matmul projects q (or packs k) for BOTH heads at once
        wq_comb = blobW[:, 0:128]
        wk_comb = [blobW[:, 128 + 128 * r : 128 + 128 * (r + 1)] for r in range(2)]
        bq_comb = blobW[:, 384:385].bitcast(F32)
        wva = blobV[:, 0:66]
        bvb = blobV[:, 66:132].bitcast(F32)

        # persistent activations (rows 64h+32r+e)
        qt_rep = big.tile([128, S], F32R, tag="qt", name="qt")
        kt_pack = big.tile([128, (NJ // 2) * 128], F32R, tag="kt", name="kt")
        # bf16: PV runs fully in bf16 (stationary pt, moving vhat)
        vhat = big.tile([128, NJ * 66], BF16, tag="vhat")

        # p-state warm-up source first so PE starts as early as possible
        zt = const.tile([128, 512], F32, tag="zt")
        nc.vector.memset(zt[:], 0.0)
        ztr = zt.bitcast(F32R)

        # bf16 zeros for the av-bank-opening dummy matmul (on Pool, in
        # parallel with the DVE memset above)
        zbf = const.tile([128, 512], BF16, tag="zbf")
        nc.gpsimd.memset(zbf[:], 0.0)

        # force the exp_and_others act table (covers identity+exp) up front
        scratch = const.tile([1, 1], F32, tag="scr")
        nc.scalar.activation(scratch[:], blobV[0:1, 0:1].bitcast(F32), AF.Exp)
        zp = pss.tile([128, 1024], F32, tag="s", name="zp")
        for i in range(5):
            nc.tensor.matmul(
                zp[:, 0:512], ztr[:, 0:128], ztr[:, 0:512], start=(i == 0), stop=(i == 4)
            )

        # ---- projection emitters (psum from the pss pool) ----
        def ps_tile(name):
            return pss.tile([128, 1024], F32, tag="s", name=name)

        def v_pair(j):
            # chunks j, j+1 into one tile (separate banks, DIFFERENT
            # stationaries - same-stationary bank pairs corrupt on HW),
            # one broadcast bias-add for both
            pv = ps_tile(f"pv{j}")
            for u in range(2):
                nc.tensor.matmul(
                    pv[:, 512 * u : 512 * u + 66],
                    xts[j // 8][:, ((j + u) % 8) * 128 : ((j + u) % 8) * 128 + 128],
                    wva,
                    start=True,
                    stop=True,
                )
            pv3 = pv[:].rearrange("p (u x) -> p u x", u=2)[:, :, 0:66]
            bv3 = bvb.rearrange("p (u x) -> p u x", u=1)
            pv3b, bv3b = bass.broadcast_tensor_aps(pv3, bv3)
            nc.vector.tensor_tensor(
                out=vhat[:, j * 66 : (j + 2) * 66].rearrange("p (u x) -> p u x", u=2),
                in0=pv3b,
                in1=bv3b,
                op=ALU.add,
            )

        def k_chunk(c):
            # pack kT for chunks 8c..8c+7, both heads: partition 64h+32(j%2)+e,
            # col 128*(j//2)+p
            pk = ps_tile(f"pk{c}")
            xg = xts[c][:].rearrange("d (j p) -> d j p", p=128)
            for r in range(2):
                nc.tensor.matmul(
                    pk[:, 0:512],
                    wk_comb[r],
                    xg[:, r:8:2, :],
                    start=(r == 0),
                    stop=(r == 1),
                )
            # k-mover on ACT so DVE keeps room for the vhat bias adds
            nc.scalar.activation(
                kt_pack[:, c * 512 : (c + 1) * 512],
                pk[:, 0:512],
                AF.Identity,
            )

        def q_chunk(n):
            pq = ps_tile(f"pq{n}")
            nc.tensor.matmul(
                pq[:, 0:512],
                wq_comb,
                xts[n // 2][:, (n % 2) * 512 : (n % 2) * 512 + 512],
                start=True,
                stop=True,
            )
            nc.scalar.activation(
                qt_rep[:, n * 512 : (n + 1) * 512],
                pq[:, 0:512],
                AF.Identity,
                bias=bq_comb,
            )

        def proj_tile(c):
            # k/q first: their movers gate the next scores chunks, while the
            # v-chunk PE work overlaps those movers
            k_chunk(c)
            q_chunk(2 * c)
            q_chunk(2 * c + 1)
            for j in range(8 * c, 8 * c + 8, 2):
                v_pair(j)

        # ---- attention ----
        # Decoupled software pipeline over "slots": slot s = 2g+half covers
        # key chunks (2g, 2g+1) x 512 queries (half). After scores(s) are
        # emitted, the exp of s-1 is issued and the PV of s-3.
        last_s = NJ - 1
        pending = deque()  # entries: [ps, s, av, h, q0, exp_pt]

        def issue_exp(ent, in_b0, split=False):
            ps, s, av, h, q0, _ = ent
            eng = (EXP_PAT_B0 if in_b0 else EXP_PAT)[s]
            if split:
                # drain shortcut: halves on both engines in parallel
                pti = work.tile([128, 1024], I16, tag="pti", name=f"pt{h}_{q0}_{s}")
                nc.scalar.activation(
                    pti[:, 0:512].bitcast(BF16), ps[:, 0:512], AF.Exp
                )
                nc.vector.tensor_scalar(
                    out=pti[:, 512:1024],
                    in0=ps[:, 512:1024],
                    scalar1=SCH_A,
                    scalar2=SCH_B,
                    op0=ALU.mult,
                    op1=ALU.add,
                )
                ent[5] = pti.bitcast(BF16)
                return
            if eng == "A":
                ptf = work.tile([128, 1024], BF16, tag="pt", name=f"pt{h}_{q0}_{s}")
                nc.scalar.activation(ptf[:], ps[:], AF.Exp)
                pt = ptf
            else:
                pti = work.tile([128, 1024], I16, tag="pti", name=f"pt{h}_{q0}_{s}")
                nc.vector.tensor_scalar(
                    out=pti[:],
                    in0=ps[:],
                    scalar1=SCH_A,
                    scalar2=SCH_B,
                    op0=ALU.mult,
                    op1=ALU.add,
                )
                pt = pti.bitcast(BF16)
            ent[5] = pt

        def issue_pv():
            ent = pending.popleft()
            if ent[5] is None:
                issue_exp(ent, False)
            _, s, av, h, q0, pt = ent
            g, half = s // 2, s % 2
            for r in range(2):
                j = 2 * g + r
                vs = vhat[:, j * 66 + h * 33 : j * 66 + h * 33 + 33]
                for q2 in range(4):
                    qc = 4 * half + q2
                    nc.tensor.matmul(
                        av[:, qc * 33 : qc * 33 + 33],
                        pt[:, 512 * r + 128 * q2 : 512 * r + 128 * q2 + 128],
                        vs,
                        start=False,
                        stop=(s == last_s and r == 1 and q2 == 3),
                        skip_group_check=True,
                    )
            if s == last_s:
                # normalize straight out of psum on DVE: reciprocal of the 8
                # ones-columns, then ONE broadcast tensor_tensor multiply
                # (stride-0 free dim on the reciprocal operand).
                rcp = outp.tile([128, 8], F32, tag="rcp", name=f"rc{h}_{q0}")
                nc.vector.reciprocal(rcp[:], av[:, 32:264:33])
                osb = outp.tile([128, 256], F32, tag="osb", name=f"ob{h}_{q0}")
                av3 = av[:, 0:264].rearrange("p (c x) -> p c x", x=33)[:, :, 0:32]
                rcp3 = rcp[:].rearrange("p (c x) -> p c x", x=1)
                av3b, rcp3b = bass.broadcast_tensor_aps(av3, rcp3)
                nc.vector.tensor_tensor(
                    out=osb[:].rearrange("p (c d) -> p c d", c=8),
                    in0=av3b,
                    in1=rcp3b,
                    op=ALU.mult,
                )
                nc.sync.dma_start(
                    OUT[h, q0 : q0 + 1024, :].rearrange("(c p) d -> p c d", c=8),
                    osb[:].rearrange("p (c d) -> p c d", c=8),
                )

        def flush_all():
            for ent in pending:
                if ent[5] is None:
                    issue_exp(ent, False)
            while pending:
                issue_pv()

        proj_tile(0)
        for h in range(2):
            for i0 in range(NQB):
                q0 = i0 * 1024
                in_b0 = h == 0 and i0 == 0
                av = psav.tile([128, 512], F32, tag="av", name=f"av{h}_{q0}")
                # open the accumulation bank: zeros over the FULL bank (512
                # cols) in one matmul, as v2 did -- partial-bank openers
                # misbehaved on hardware
                nc.tensor.matmul(
                    av[:, 0:512],
                    zbf[:, 0:128],
                    zbf[:, 0:512],
                    start=True,
                    stop=False,
                    skip_group_check=True,
                )
                for s in range(NJ):
                    g, half = s // 2, s % 2
                    qh0 = q0 + 512 * half
                    ps = pss.tile([128, 1024], F32, tag="s", name=f"s{h}_{q0}_{s}")
                    # slot tile: key chunks (2g, 2g+1) x 512 queries, two
                    # row-band matmuls as in v2 (one psum bank each)
                    for r in range(2):
                        off = 64 * h + 32 * r
                        nc.tensor.matmul(
                            ps[:, 512 * r : 512 * (r + 1)],
                            kt_pack[off : off + 32, g * 128 : g * 128 + 128],
                            qt_rep[off : off + 32, qh0 : qh0 + 512],
                            start=True,
                            stop=True,
                            tile_position=(off, 0),
                        )
                    pending.append([ps, s, av, h, q0, None])
                    tail_split = h == 1 and i0 == NQB - 1 and s >= NJ - 2
                    if pending[-1][5] is None:
                        issue_exp(pending[-1], in_b0, split=tail_split)
                    if len(pending) >= 5:
                        issue_pv()
                    # interleave remaining xt-tile projections into block 0;
                    # pre-issue pending exps so the 11 psum allocations never
                    # WAR-wait on a not-yet-emitted exp (emission deadlock) —
                    # the PV backlog itself can stay pending.
                    if in_b0 and s in (1, 9, 17):
                        c = s // 8 + 1
                        if c < NT:
                            for ent in pending:
                                if ent[5] is None:
                                    issue_exp(ent, True)
                            proj_tile(c)
        flush_all()

    nc.compile()
    return nc


def _host_prep(x, Wq, bq, Wk, bk, Wv, bv, S):
    """Per-core input maps."""
    in_maps = []
    for c in range(NCORES):
        b, hp = c // 2, c % 2
        h0, h1 = 2 * hp, 2 * hp + 1
        xt = np.ascontiguousarray(x[b].T).astype(np.float32)  # [128, S]
        blob = np.zeros((128, 518), np.float32)
        for i, hh in enumerate((h0, h1)):
            wq_h = Wq[hh * 32 : (hh + 1) * 32, :]  # [32, 128]
            wk_h = Wk[hh * 32 : (hh + 1) * 32, :]
            # combined-head layout: output partition p = 64i + 32r + e
            blob[:, 64 * i : 64 * (i + 1)] = np.tile(wq_h.T, (1, 2))
            for r in range(2):
                off = 128 + 128 * r + 64 * i + 32 * r
                blob[:, off : off + 32] = wk_h.T
            blob[64 * i : 64 * (i + 1), 384] = np.tile(bq[hh * 32 : (hh + 1) * 32], 2)
            blob[:, 386 + 33 * i : 386 + 33 * i + 32] = Wv[hh * 32 : (hh + 1) * 32, :].T
            blob[:, 452 + 33 * i : 452 + 33 * i + 32] = bv[hh * 32 : (hh + 1) * 32][None, :]
            blob[:, 452 + 33 * i + 32] = 1.0
        in_maps.append({"XT": xt, "WBLOB": blob})
    return in_maps


def _unshard(results, S):
    out = np.empty((B, S, D), np.float32)
    for c in range(NCORES):
        b, hp = c // 2, c % 2
        oc = results[c]["OUT"]  # [2, S, 32]
        for hl in range(2):
            hh = 2 * hp + hl
            out[b, :, hh * 32 : (hh + 1) * 32] = oc[hl]
    return out


def _run_once(args):
    x, Wq, bq, Wk, bk, Wv, bv = args
    S = x.shape[1]
    if S not in _built:
        _built[S] = build_nc(S)
    nc = _built[S]
    in_maps = _host_prep(x, Wq, bq, Wk, bk, Wv, bv, S)
    res = bass_utils.run_bass_kernel_spmd(nc, in_maps, core_ids=list(range(NCORES)))
    return _unshard(res.results, S)


def _subproc_entry(args):
    return _run_once(args)


def kernel(x, Wq, bq, Wk, bk, Wv, bv):
    args = tuple(
        np.asarray(a, dtype=np.float32) for a in (x, Wq, bq, Wk, bk, Wv, bv)
    )
    # The axon/NRT stack occasionally fails a first dispatch with
    # NRT_EXEC_UNIT_UNRECOVERABLE (device auto-recovers). Retry in-process,
    # then in a fresh spawned process (compile caches make that cheap).
    try:
        return _run_once(args)
    except Exception:
        try:
            return _run_once(args)
        except Exception:
            import multiprocessing as mp

            ctx = mp.get_context("spawn")
            with ctx.Pool(1) as pool:
                return pool.apply(_subproc_entry, (args,))
